# Initial kernel scaffold
#
"""Trainium2 Bass kernel for nn_DGraph (gnn_message_passing).

Computation per batch b (reference):
    energy    = x @ x^T                              [C, C]
    attention = softmax(rowmax(energy) - energy)     [C, C]  (== softmax(-energy) row-wise)
    x_glb     = gamma * (attention @ x) + x          [C, N]
    xc        = concat([x_glb, x], channel)          [2C, N]
    adj       = sigmoid(w_adj @ xc + b_adj)          [N, N]
    y         = lrelu(x @ adj)                       [C, N]
    y         = lrelu(w_dyn @ y + b_dyn)             [OUT, N]

Sharding: data-parallel over batch B=8, one batch per NeuronCore (8 cores).
Weights are replicated. Everything is fused on-chip: the [N, N] adj matrix
is never materialized in HBM - it is produced in PSUM chunk-by-chunk,
sigmoided into SBUF, and immediately consumed by the y matmul.

Matmuls use float32r (full fp32 storage; reduced-precision single-pass PE
multiply) which streams at 1 row/cycle for >=256-wide moving operands,
4x faster than plain fp32.
"""

import numpy as np


def _ensure_concourse():
    try:
        import concourse.bass  # noqa: F401
    except ImportError:
        import sys

        for p in ("/opt/trn_rl_repo", "/root/.axon_site/_ro/trn_rl_repo"):
            if p not in sys.path:
                sys.path.insert(0, p)


_ensure_concourse()

B, C, OUT, N = 8, 128, 128, 2048
P = 128
KO = N // P  # 16 k-chunks of adj rows
NTILE = 512
NT = N // NTILE  # 4 column tiles
NEG_SLOPE = 0.2

_cache = {}


def _build_program():
    if "nc" in _cache:
        return _cache["nc"]

    from contextlib import ExitStack

    import concourse.bass as bass  # noqa: F401
    import concourse.tile as tile
    from concourse import bacc, mybir
    from concourse.masks import make_identity

    f32 = mybir.dt.float32
    f32r = mybir.dt.float32r
    AF = mybir.ActivationFunctionType
    ALU = mybir.AluOpType

    nc = bacc.Bacc(
        "TRN2",
        target_bir_lowering=False,
        debug=False,
        enable_asserts=False,
        num_devices=B,
    )

    x_d = nc.dram_tensor("x", [P, N], f32, kind="ExternalInput").ap()
    wadjT_d = nc.dram_tensor("w_adjT", [2 * C, N], f32, kind="ExternalInput").ap()
    badj_d = nc.dram_tensor("b_adj2", [P, KO], f32, kind="ExternalInput").ap()
    wdynT_d = nc.dram_tensor("w_dynT", [C, OUT], f32, kind="ExternalInput").ap()
    bdyn_d = nc.dram_tensor("b_dyn2", [P, 1], f32, kind="ExternalInput").ap()
    gam_d = nc.dram_tensor("gamma2", [P, 1], f32, kind="ExternalInput").ap()
    y_d = nc.dram_tensor("y", [P, N], f32, kind="ExternalOutput").ap()

    with tile.TileContext(nc) as tc, ExitStack() as ctx:
        singles = ctx.enter_context(tc.tile_pool(name="singles", bufs=1))

        x_sb = singles.tile([P, N], f32)
        xT_sb = singles.tile([P, KO, P], f32)
        xglb_sb = singles.tile([P, N], f32)
        wadj_sb = singles.tile([P, 2, N], f32)
        wdyn_sb = singles.tile([P, OUT], f32)
        badj_sb = singles.tile([P, KO], f32)
        bdyn_sb = singles.tile([P, 1], f32)
        gam_sb = singles.tile([P, 1], f32)
        ident = singles.tile([P, P], f32)
        p_sb = singles.tile([P, P], f32)
        att_sb = singles.tile([P, P], f32)
        attT_sb = singles.tile([P, P], f32)
        rowmin = singles.tile([P, 1], f32)
        rowsum = singles.tile([P, 1], f32)
        rinv = singles.tile([P, 1], f32)

        nc.sync.dma_start(x_sb, x_d)
        nc.sync.dma_start(wadj_sb[:, 0], wadjT_d[0:P])
        nc.sync.dma_start(wadj_sb[:, 1], wadjT_d[P : 2 * P])
        nc.sync.dma_start(wdyn_sb, wdynT_d)
        nc.sync.dma_start(badj_sb, badj_d)
        nc.sync.dma_start(bdyn_sb, bdyn_d)
        nc.sync.dma_start(gam_sb, gam_d)
        make_identity(nc, ident)

        # ---- Phase A: attention (energy -> softmax -> att @ x -> x_glb) ----
        with tc.tile_pool(name="psumA", bufs=2, space="PSUM") as psumA:
            # xT[n, c] chunks via PE transpose (fp32 has no DMA transpose)
            for ko in range(KO):
                pt = psumA.tile([P, P], f32, tag="pt")
                nc.tensor.transpose(pt, x_sb[:, ko * P : (ko + 1) * P], ident)
                nc.vector.tensor_copy(xT_sb[:, ko], pt)

            # energy[c, d] = sum_n x[c, n] x[d, n]
            pe_e = psumA.tile([P, P], f32, tag="pe")
            for ko in range(KO):
                nc.tensor.matmul(
                    pe_e,
                    xT_sb[:, ko],
                    xT_sb[:, ko],
                    start=(ko == 0),
                    stop=(ko == KO - 1),
                )

            # softmax(rowmax - energy) == exp(rowmin - energy) / sum
            nc.vector.tensor_reduce(
                rowmin, pe_e, axis=mybir.AxisListType.X, op=ALU.min
            )
            nc.scalar.activation(
                p_sb, pe_e, AF.Exp, bias=rowmin, scale=-1.0, accum_out=rowsum
            )
            nc.vector.reciprocal(rinv, rowsum)
            nc.vector.tensor_scalar_mul(att_sb, p_sb, rinv)

            # attT[d, c] for use as stationary operand
            ptA = psumA.tile([P, P], f32, tag="pt")
            nc.tensor.transpose(ptA, att_sb, ident)
            nc.vector.tensor_copy(attT_sb, ptA)

        # x_glb = gamma * (att @ x) + x
        with tc.tile_pool(name="psumG", bufs=2, space="PSUM") as psumG:
            for nt in range(NT):
                sl = slice(nt * NTILE, (nt + 1) * NTILE)
                pg = psumG.tile([P, NTILE], f32, tag="pg")
                nc.tensor.matmul(
                    pg,
                    attT_sb.bitcast(f32r),
                    x_sb[:, sl].bitcast(f32r),
                    start=True,
                    stop=True,
                )
                nc.vector.scalar_tensor_tensor(
                    out=xglb_sb[:, sl],
                    in0=pg,
                    scalar=gam_sb,
                    in1=x_sb[:, sl],
                    op0=ALU.mult,
                    op1=ALU.add,
                )

        # ---- Phase B: adj = sigmoid(w_adj @ [x_glb; x] + b), fused with
        #      y = lrelu(x @ adj);  y = lrelu(w_dyn @ y + b_dyn) ----
        adj_pool = ctx.enter_context(tc.tile_pool(name="adj", bufs=3))
        y1_pool = ctx.enter_context(tc.tile_pool(name="y1", bufs=2))
        out_pool = ctx.enter_context(tc.tile_pool(name="outp", bufs=3))
        psum_adj = ctx.enter_context(tc.tile_pool(name="padj", bufs=2, space="PSUM"))
        psum_y = ctx.enter_context(tc.tile_pool(name="py", bufs=2, space="PSUM"))
        psum_f = ctx.enter_context(tc.tile_pool(name="pf", bufs=2, space="PSUM"))

        for nt in range(NT):
            sl = slice(nt * NTILE, (nt + 1) * NTILE)
            py = psum_y.tile([P, NTILE], f32, tag="py")
            for ko in range(KO):
                ks = slice(ko * P, (ko + 1) * P)
                pa = psum_adj.tile([P, NTILE], f32, tag="pa")
                nc.tensor.matmul(
                    pa,
                    wadj_sb[:, 0, ks].bitcast(f32r),
                    xglb_sb[:, sl].bitcast(f32r),
                    start=True,
                    stop=False,
                )
                nc.tensor.matmul(
                    pa,
                    wadj_sb[:, 1, ks].bitcast(f32r),
                    x_sb[:, sl].bitcast(f32r),
                    start=False,
                    stop=True,
                )
                adj = adj_pool.tile([P, NTILE], f32, tag="adj")
                nc.scalar.activation(
                    adj, pa, AF.Sigmoid, bias=badj_sb[:, ko : ko + 1]
                )
                nc.tensor.matmul(
                    py,
                    xT_sb[:, ko].bitcast(f32r),
                    adj.bitcast(f32r),
                    start=(ko == 0),
                    stop=(ko == KO - 1),
                )

            y1 = y1_pool.tile([P, NTILE], f32, tag="y1")
            nc.scalar.activation(y1, py, AF.Lrelu, alpha=NEG_SLOPE)

            pf = psum_f.tile([P, NTILE], f32, tag="pf")
            nc.tensor.matmul(
                pf, wdyn_sb.bitcast(f32r), y1.bitcast(f32r), start=True, stop=True
            )
            ot = out_pool.tile([P, NTILE], f32, tag="ot")
            nc.scalar.activation(ot, pf, AF.Lrelu, bias=bdyn_sb, alpha=NEG_SLOPE)
            nc.sync.dma_start(y_d[:, sl], ot)

    nc.compile()
    _cache["nc"] = nc
    return nc


def _prep_inputs(x, w_adj, b_adj, w_dyn, b_dyn, gamma):
    x = np.ascontiguousarray(np.asarray(x, dtype=np.float32))
    wadjT = np.ascontiguousarray(np.asarray(w_adj, dtype=np.float32).T)
    badj2 = np.ascontiguousarray(
        np.asarray(b_adj, dtype=np.float32).reshape(KO, P).T
    )
    wdynT = np.ascontiguousarray(np.asarray(w_dyn, dtype=np.float32).T)
    bdyn2 = np.ascontiguousarray(np.asarray(b_dyn, dtype=np.float32).reshape(P, 1))
    gam2 = np.ascontiguousarray(
        np.broadcast_to(np.asarray(gamma, dtype=np.float32).reshape(1, 1), (P, 1))
    )
    return [
        dict(
            x=x[b],
            w_adjT=wadjT,
            b_adj2=badj2,
            w_dynT=wdynT,
            b_dyn2=bdyn2,
            gamma2=gam2,
        )
        for b in range(B)
    ]


def kernel(x, w_adj, b_adj, w_dyn, b_dyn, gamma):
    nc = _build_program()
    from concourse.bass_utils import run_bass_kernel_spmd

    in_maps = _prep_inputs(x, w_adj, b_adj, w_dyn, b_dyn, gamma)
    res = run_bass_kernel_spmd(nc, in_maps, core_ids=list(range(B)))
    return np.stack([res.results[b]["y"] for b in range(B)], axis=0).astype(
        np.float32
    )


# revision 43
# speedup vs baseline: 1.1429x; 1.1429x over previous
"""Trainium2 Bass kernel for nn_DGraph (gnn_message_passing).

Computation per batch b (reference):
    energy    = x @ x^T                              [C, C]
    attention = softmax(rowmax(energy) - energy)     [C, C]  (== softmax(-energy) row-wise)
    x_glb     = gamma * (attention @ x) + x          [C, N]
    xc        = concat([x_glb, x], channel)          [2C, N]
    adj       = sigmoid(w_adj @ xc + b_adj)          [N, N]
    y         = lrelu(x @ adj)                       [C, N]
    y         = lrelu(w_dyn @ y + b_dyn)             [OUT, N]

Sharding: data-parallel over batch B=8, one batch per NeuronCore (8 cores).
Weights are replicated. Everything is fused on-chip: the [N, N] adj matrix
is never materialized in HBM - it is produced in PSUM chunk-by-chunk,
sigmoided into SBUF, and immediately consumed by the y matmul.

Matmuls use float32r (full fp32 storage; reduced-precision single-pass PE
multiply) which streams at 1 row/cycle for >=256-wide moving operands,
4x faster than plain fp32.
"""

import numpy as np


def _ensure_concourse():
    try:
        import concourse.bass  # noqa: F401
    except ImportError:
        import sys

        for p in ("/opt/trn_rl_repo", "/root/.axon_site/_ro/trn_rl_repo"):
            if p not in sys.path:
                sys.path.insert(0, p)


_ensure_concourse()

B, C, OUT, N = 8, 128, 128, 2048
P = 128
KO = N // P  # 16 k-chunks of adj rows
NTILE = 512
NT = N // NTILE  # 4 column tiles
NEG_SLOPE = 0.2

_cache = {}


def _build_program(use_act_lrelu=True):
    # use_act_lrelu=True: leaky-relu via ScalarE Prelu (verified bit-exact on
    # HW, alpha honored; not implemented in CoreSim). False: DVE max-pair
    # fallback, used by the simulator-based test only.
    key = ("nc", use_act_lrelu)
    if key in _cache:
        return _cache[key]

    from contextlib import ExitStack

    import concourse.bass as bass  # noqa: F401
    import concourse.tile as tile
    from concourse import bacc, mybir
    from concourse.masks import make_identity

    f32 = mybir.dt.float32
    f32r = mybir.dt.float32r
    AF = mybir.ActivationFunctionType
    ALU = mybir.AluOpType

    nc = bacc.Bacc(
        "TRN2",
        target_bir_lowering=False,
        debug=False,
        enable_asserts=False,
        num_devices=B,
    )

    xr_d = nc.dram_tensor("xr", [P, N], f32r, kind="ExternalInput").ap()
    wadjT_d = nc.dram_tensor("w_adjT", [2 * C, N], f32r, kind="ExternalInput").ap()
    smalls_d = nc.dram_tensor(
        "smalls", [P, 18 + OUT], f32r, kind="ExternalInput"
    ).ap()
    y_d = nc.dram_tensor("y", [P, N], f32, kind="ExternalOutput").ap()

    with tile.TileContext(nc) as tc, ExitStack() as ctx:
        singles = ctx.enter_context(tc.tile_pool(name="singles", bufs=1))

        # x lives once in SBUF as f32r (same bits as f32); fp32 consumers
        # (PE transpose, DVE epilogues) read it through a bitcast view.
        x_sbr = singles.tile([P, N], f32r)
        x_sb = x_sbr.bitcast(f32)
        xT_sb = singles.tile([P, KO, P], f32r)
        xglb_sb = singles.tile([P, N], f32r)
        wadj_sb = singles.tile([P, 2, N], f32r)
        # biases/gamma/wdyn arrive as one coalesced [P, 18+OUT] f32r tensor
        # (separate [128, tiny] DMAs shatter into 128 sub-100B packets each
        # and clog the queue). fp32 consumers read bitcast views.
        smalls_sb = singles.tile([P, 18 + OUT], f32r)
        badj_sb = smalls_sb[:, 0:KO].bitcast(f32)
        bdyn_sb = smalls_sb[:, 16:17].bitcast(f32)
        gam_sb = smalls_sb[:, 17:18].bitcast(f32)
        wdyn_sb = smalls_sb[:, 18 : 18 + OUT]
        ident = singles.tile([P, P], f32)
        s_sb = singles.tile([P, P], f32)
        oms_sb = singles.tile([P, P], f32)
        roms_sb = singles.tile([P, P], f32)
        p_sb = singles.tile([P, P], f32)
        attT_sb = singles.tile([P, P], f32r)
        rowmin = singles.tile([P, 1], f32)
        rowsum = singles.tile([P, 1], f32)
        rinv = singles.tile([P, 1], f32)

        # Input DMA: few descriptors with 4-8KB-contiguous partition lines
        # (small chunks shatter into 2KB packets and tank the queue to
        # ~60 GB/s). x first, split across both HWDGE rings; then the
        # weight needed first (wadj x-half), then the rest.
        half = N // 2
        nc.sync.dma_start(x_sbr[:, :half], xr_d[:, :half])
        nc.scalar.dma_start(x_sbr[:, half:], xr_d[:, half:])
        nc.scalar.dma_start(wadj_sb[:, 1], wadjT_d[P : 2 * P])
        nc.sync.dma_start(wadj_sb[:, 0], wadjT_d[0:P])
        nc.sync.dma_start(smalls_sb, smalls_d)
        make_identity(nc, ident)

        # ---- Phase A: attention (energy -> softmax -> att @ x -> x_glb) ----
        with tc.tile_pool(name="psumA", bufs=2, space="PSUM") as psumA:
            # xT[n, c] chunks via PE transpose (fp32 has no DMA transpose)
            for ko in range(KO):
                pt = psumA.tile([P, P], f32, tag="pt", bufs=4)
                nc.tensor.transpose(pt, x_sb[:, ko * P : (ko + 1) * P], ident)
                nc.vector.tensor_copy(xT_sb[:, ko], pt)

            # energy[c, d] = sum_n x[c, n] x[d, n]
            pe_e = psumA.tile([P, P], f32, tag="pe")
            for ko in range(KO):
                nc.tensor.matmul(
                    pe_e,
                    xT_sb[:, ko],
                    xT_sb[:, ko],
                    start=(ko == 0),
                    stop=(ko == KO - 1),
                )

            # softmax(rowmax - energy) == exp(z)/sum, z = rowmin - energy <= 0.
            # exp via the sigmoid table (exp(z) = s/(1-s), s = sigmoid(z)) so
            # the whole kernel needs only ONE ACT table set (sigmoid; the
            # leaky-relus are in every set) - saves two 1.3us table loads.
            nc.vector.tensor_reduce(
                rowmin, pe_e, axis=mybir.AxisListType.X, op=ALU.min
            )
            nc.scalar.activation(s_sb, pe_e, AF.Sigmoid, bias=rowmin, scale=-1.0)
            nc.vector.tensor_scalar(
                oms_sb, s_sb, -1.0, 1.0, op0=ALU.mult, op1=ALU.add
            )
            nc.vector.reciprocal(roms_sb, oms_sb)
            nc.vector.tensor_tensor(out=p_sb, in0=s_sb, in1=roms_sb, op=ALU.mult)

            # Normalization is deferred: p (unnormalized) goes through the
            # att @ x matmul; gamma/rowsum is folded into the x_glb epilogue
            # scale. This takes the reduce/reciprocal off the serial chain
            # (they run in parallel with the transpose and matmul below).
            nc.vector.tensor_reduce(
                rowsum, p_sb, axis=mybir.AxisListType.X, op=ALU.add
            )
            nc.vector.reciprocal(rinv, rowsum)
            grinv = singles.tile([P, 1], f32)
            nc.vector.tensor_tensor(out=grinv, in0=rinv, in1=gam_sb, op=ALU.mult)

            # pT[d, c] for use as stationary operand
            ptA = psumA.tile([P, P], f32, tag="pt", bufs=4)
            nc.tensor.transpose(ptA, p_sb, ident)
            nc.vector.tensor_copy(attT_sb, ptA)

        # x_glb = (gamma/rowsum) * (p @ x) + x
        with tc.tile_pool(name="psumG", bufs=2, space="PSUM") as psumG:
            for nt in range(NT):
                sl = slice(nt * NTILE, (nt + 1) * NTILE)
                pg = psumG.tile([P, NTILE], f32, tag="pg")
                nc.tensor.matmul(
                    pg, attT_sb, x_sbr[:, sl], start=True, stop=True
                )
                nc.vector.scalar_tensor_tensor(
                    out=xglb_sb[:, sl],
                    in0=pg,
                    scalar=grinv,
                    in1=x_sb[:, sl],
                    op0=ALU.mult,
                    op1=ALU.add,
                )

        # ---- Phase B: adj = sigmoid(w_adj @ [x_glb; x] + b), fused with
        #      y = lrelu(x @ adj);  y = lrelu(w_dyn @ y + b_dyn) ----
        adj_pool = ctx.enter_context(tc.tile_pool(name="adj", bufs=3))
        y1_pool = ctx.enter_context(tc.tile_pool(name="y1", bufs=2))
        out_pool = ctx.enter_context(tc.tile_pool(name="outp", bufs=3))
        # PSUM budget (8 banks): padj 2 bufs x 2 banks = 4, py 2 tags x 1 = 2,
        # pf 2 -> 8 total.
        psum_adj = ctx.enter_context(tc.tile_pool(name="padj", bufs=2, space="PSUM"))
        psum_y = ctx.enter_context(tc.tile_pool(name="py", bufs=1, space="PSUM"))
        psum_f = ctx.enter_context(tc.tile_pool(name="pf", bufs=2, space="PSUM"))

        # Process n-tiles in pairs: one [P, 2, NTILE] sigmoid per k-chunk
        # (halves ACT per-instruction overhead), and run the y-accumulation
        # matmuls one k-iteration behind the adj matmuls so the PE never
        # waits on the current sigmoid.
        NPAIR = 2
        for pr in range(NT // NPAIR):
            nts = [pr * NPAIR + j for j in range(NPAIR)]
            sls = [slice(nt * NTILE, (nt + 1) * NTILE) for nt in nts]
            pys = [
                psum_y.tile([P, NTILE], f32, tag=f"py{j}", name=f"py{j}_{pr}")
                for j in range(NPAIR)
            ]

            def y_accum(adj_t, ko):
                for j in range(NPAIR):
                    nc.tensor.matmul(
                        pys[j],
                        xT_sb[:, ko],
                        adj_t[:, j],
                        start=(ko == 0),
                        stop=(ko == KO - 1),
                    )

            prev = None
            for ko in range(KO):
                ks = slice(ko * P, (ko + 1) * P)
                pa = psum_adj.tile([P, NPAIR, NTILE], f32, tag="pa")
                for j in range(NPAIR):
                    ADJ_REORDER = True
                    if ADJ_REORDER:
                        # x-half first: it does not depend on the attention
                        # phase, so the scheduler can run it early and keep
                        # the PE warm while phase A's serial chain finishes.
                        nc.tensor.matmul(
                            pa[:, j], wadj_sb[:, 1, ks], x_sbr[:, sls[j]],
                            start=True, stop=False,
                        )
                        nc.tensor.matmul(
                            pa[:, j], wadj_sb[:, 0, ks], xglb_sb[:, sls[j]],
                            start=False, stop=True,
                        )
                    else:
                        nc.tensor.matmul(
                            pa[:, j], wadj_sb[:, 0, ks], xglb_sb[:, sls[j]],
                            start=True, stop=False,
                        )
                        nc.tensor.matmul(
                            pa[:, j], wadj_sb[:, 1, ks], x_sbr[:, sls[j]],
                            start=False, stop=True,
                        )
                adj = adj_pool.tile([P, NPAIR, NTILE], f32r, tag="adj")
                nc.scalar.activation(
                    adj, pa, AF.Sigmoid, bias=badj_sb[:, ko : ko + 1]
                )
                if prev is not None:
                    y_accum(*prev)
                prev = (adj, ko)
            y_accum(*prev)

            for j in range(NPAIR):
                y1 = y1_pool.tile([P, NTILE], f32r, tag="y1")
                if use_act_lrelu:
                    # Prelu == leaky_relu with honored alpha (HW-verified)
                    nc.scalar.activation(y1, pys[j], AF.Prelu, alpha=NEG_SLOPE)
                else:
                    # Sim fallback: lrelu(z) = max(z, 0.2 z); only one PSUM
                    # operand per DVE instruction, so scale to SBUF first.
                    y1s = y1_pool.tile([P, NTILE], f32, tag="y1s")
                    nc.vector.tensor_scalar_mul(y1s, pys[j], NEG_SLOPE)
                    nc.vector.tensor_tensor(
                        out=y1, in0=y1s, in1=pys[j], op=ALU.max
                    )

                pf = psum_f.tile([P, NTILE], f32, tag="pf")
                nc.tensor.matmul(pf, wdyn_sb, y1, start=True, stop=True)
                ot = out_pool.tile([P, NTILE], f32, tag="ot")
                if use_act_lrelu:
                    nc.scalar.activation(
                        ot, pf, AF.Prelu, bias=bdyn_sb, alpha=NEG_SLOPE
                    )
                else:
                    zt = out_pool.tile([P, NTILE], f32, tag="zt")
                    nc.vector.tensor_scalar_add(zt, pf, bdyn_sb)
                    nc.vector.scalar_tensor_tensor(
                        out=ot, in0=zt, scalar=NEG_SLOPE, in1=zt,
                        op0=ALU.mult, op1=ALU.max,
                    )
                nc.sync.dma_start(y_d[:, sls[j]], ot)

    nc.compile()
    _cache[key] = nc
    return nc


def _prep_inputs(x, w_adj, b_adj, w_dyn, b_dyn, gamma):
    x = np.ascontiguousarray(np.asarray(x, dtype=np.float32))
    wadjT = np.ascontiguousarray(np.asarray(w_adj, dtype=np.float32).T)
    smalls = np.empty((P, 18 + OUT), dtype=np.float32)
    smalls[:, 0:KO] = np.asarray(b_adj, dtype=np.float32).reshape(KO, P).T
    smalls[:, 16] = np.asarray(b_dyn, dtype=np.float32)
    smalls[:, 17] = np.float32(np.asarray(gamma, dtype=np.float32).reshape(()))
    smalls[:, 18:] = np.asarray(w_dyn, dtype=np.float32).T
    return [
        dict(xr=x[b], w_adjT=wadjT, smalls=smalls) for b in range(B)
    ]


def kernel(x, w_adj, b_adj, w_dyn, b_dyn, gamma):
    nc = _build_program()
    from concourse.bass_utils import run_bass_kernel_spmd

    in_maps = _prep_inputs(x, w_adj, b_adj, w_dyn, b_dyn, gamma)
    res = run_bass_kernel_spmd(nc, in_maps, core_ids=list(range(B)))
    return np.stack([res.results[b]["y"] for b in range(B)], axis=0).astype(
        np.float32
    )
